# revision 3
# baseline (speedup 1.0000x reference)
"""GQA kernel for Trainium2: B=2,T=2048,E=2048,G=4,QPG=4,D=128, causal + sinusoidal PE.

Sharding: one core per (batch, kv-group) pair = 2*4 = 8 cores.
Each core computes q/k/v projections for its group, attention for its 4 query
heads, and a partial output projection (its group's 512 columns of wo);
partials are summed on the host.

Layout strategy (per core):
  - host passes x^T (f32r), so projections run as lhsT=weight-tile, rhs=xT-tile
    producing Q^T/K^T/V^T [d, t] directly.
  - scores are computed transposed: S^T[tk, tq] = K^T_tile.T @ Q^T, softmax'd
    without max subtraction (scores bounded, verified), exp'd into bf16 P^T
    tiles that feed the PV matmul directly as lhsT.
  - denominator comes free via a ones-column appended to V (N=129).
  - attention output [tq, d] is normalized via per-partition scale, then
    PE-transposed to [d, tq] to feed the wo matmul as lhsT.
"""
import sys

sys.path.insert(0, "/opt/trn_rl_repo")

import math
import numpy as np

B, T, E = 2, 2048, 2048
G, QPG, D = 4, 4, 128
NQ = QPG * D          # 512 q columns per group
NKV = 2 * D           # 256 kv columns per group
TT = T // 128         # 16 t-tiles
TB = T // 512         # 4 t-blocks
NE = E // 128         # 16 e-tiles
ISD = 1.0 / math.sqrt(D)

_compiled = None


def _build():
    from concourse import bacc, tile, mybir

    f32 = mybir.dt.float32
    f32r = mybir.dt.float32r
    bf16 = mybir.dt.bfloat16
    ADD = mybir.AluOpType.add
    MULT = mybir.AluOpType.mult
    EXP = mybir.ActivationFunctionType.Exp
    COPY = mybir.ActivationFunctionType.Copy
    IDENT = mybir.ActivationFunctionType.Identity

    nc = bacc.Bacc("TRN2", target_bir_lowering=False, debug=False, num_devices=8)

    xt_d = nc.dram_tensor("xt", [E, T], f32r, kind="ExternalInput")        # x^T
    wq_d = nc.dram_tensor("wq", [E, NQ], f32r, kind="ExternalInput")       # group slice
    wkv_d = nc.dram_tensor("wkv", [E, NKV], f32r, kind="ExternalInput")    # group slice
    wo_d = nc.dram_tensor("wo", [NQ, E], f32r, kind="ExternalInput")       # group slice
    pet_d = nc.dram_tensor("pet", [D, T], f32, kind="ExternalInput")       # pe^T
    bq_d = nc.dram_tensor("bq", [D, QPG], f32, kind="ExternalInput")       # col h
    bk_d = nc.dram_tensor("bk", [D, 1], f32, kind="ExternalInput")
    bv_d = nc.dram_tensor("bv", [D, 1], f32, kind="ExternalInput")
    msk_d = nc.dram_tensor("msk", [4, 128, 512], bf16, kind="ExternalInput")
    idf_d = nc.dram_tensor("idf", [128, 128], f32, kind="ExternalInput")
    idb_d = nc.dram_tensor("idb", [128, 128], bf16, kind="ExternalInput")
    ones_d = nc.dram_tensor("ones1", [128, 1], bf16, kind="ExternalInput")
    out_d = nc.dram_tensor("out", [T, E], f32, kind="ExternalOutput")

    with tile.TileContext(nc) as tc:
        with tc.tile_pool(name="persist", bufs=1) as pp:
            # ---- persistent tiles ----
            pet = pp.tile([D, T], f32)
            nc.sync.dma_start(pet[:], pet_d[:])
            bq = pp.tile([D, QPG], f32)
            nc.sync.dma_start(bq[:], bq_d[:])
            bk = pp.tile([D, 1], f32)
            nc.sync.dma_start(bk[:], bk_d[:])
            bv = pp.tile([D, 1], f32)
            nc.sync.dma_start(bv[:], bv_d[:])
            idf = pp.tile([128, 128], f32)
            nc.sync.dma_start(idf[:], idf_d[:])
            idb = pp.tile([128, 128], bf16)
            nc.sync.dma_start(idb[:], idb_d[:])
            ones1 = pp.tile([128, 1], bf16)
            nc.sync.dma_start(ones1[:], ones_d[:])
            msk = [pp.tile([128, 512], bf16, name=f"msk{j}", tag=f"msk{j}") for j in range(4)]
            for j in range(4):
                nc.sync.dma_start(msk[j][:], msk_d[j])

            qt = [pp.tile([128, T], f32r, name=f"qt{h}", tag=f"qt{h}") for h in range(QPG)]
            kt = pp.tile([128, T], f32r)
            vext = [pp.tile([128, 132], bf16, name=f"vx{i}", tag=f"vx{i}") for i in range(TT)]
            at = [pp.tile([128, T], f32r, name=f"at{h}", tag=f"at{h}") for h in range(QPG)]
            wo_sb = [pp.tile([128, E], f32r, name=f"wo{h}", tag=f"wo{h}") for h in range(QPG)]
            for h in range(QPG):
                nc.sync.dma_start(wo_sb[h][:], wo_d[h * 128:(h + 1) * 128, :])

            # ---- phase 1: projections ----
            with (
                tc.tile_pool(name="p1", bufs=1) as p1,
                tc.tile_pool(name="p1x", bufs=4) as p1x,
                tc.tile_pool(name="psA", bufs=1, space="PSUM") as psA,
                tc.tile_pool(name="ps1b", bufs=2, space="PSUM") as ps1b,
            ):
                wq_sb = [p1.tile([128, NQ], f32r, name=f"wq{e}", tag=f"wq{e}") for e in range(NE)]
                wkv_sb = [p1.tile([128, NKV], f32r, name=f"wkv{e}", tag=f"wkv{e}") for e in range(NE)]
                for e in range(NE):
                    nc.sync.dma_start(wq_sb[e][:], wq_d[e * 128:(e + 1) * 128, :])
                    nc.sync.dma_start(wkv_sb[e][:], wkv_d[e * 128:(e + 1) * 128, :])

                for tb in range(TB):
                    ts = slice(tb * 512, (tb + 1) * 512)
                    qt_ps = psA.tile([128, 4 * 512], f32, name="qt_ps", tag="qt_ps")
                    kt_ps = psA.tile([128, 512], f32, name="kt_ps", tag="kt_ps")
                    vt_ps = psA.tile([128, 512], f32, name="vt_ps", tag="vt_ps")
                    for e in range(NE):
                        xt_t = p1x.tile([128, 512], f32r, name="xt", tag="xt")
                        nc.sync.dma_start(xt_t[:], xt_d[e * 128:(e + 1) * 128, ts])
                        st = e == 0
                        sp = e == NE - 1
                        for h in range(QPG):
                            nc.tensor.matmul(
                                qt_ps[:, h * 512:(h + 1) * 512],
                                wq_sb[e][:, h * 128:(h + 1) * 128],
                                xt_t[:], start=st, stop=sp,
                            )
                        nc.tensor.matmul(kt_ps[:], wkv_sb[e][:, 0:128], xt_t[:], start=st, stop=sp)
                        nc.tensor.matmul(vt_ps[:], wkv_sb[e][:, 128:256], xt_t[:], start=st, stop=sp)
                    # drain: bias (in-place on psum) then += pe^T -> sbuf f32r
                    for h in range(QPG):
                        sl = qt_ps[:, h * 512:(h + 1) * 512]
                        nc.vector.tensor_tensor(sl, sl, bq[:, h:h + 1].to_broadcast([128, 512]), ADD)
                        nc.vector.tensor_tensor(qt[h][:, ts], sl, pet[:, ts], ADD)
                    nc.vector.tensor_tensor(kt_ps[:], kt_ps[:], bk[:].to_broadcast([128, 512]), ADD)
                    nc.vector.tensor_tensor(kt[:, ts], kt_ps[:], pet[:, ts], ADD)
                    # v: bias then cast to bf16, then transpose each 128-tile
                    vtb = p1.tile([128, 512], bf16, name="vtb", tag="vtb")
                    nc.scalar.activation(vtb[:], vt_ps[:], IDENT, bias=bv[:], scale=1.0)
                    for i in range(4):
                        ti = tb * 4 + i
                        vtp = ps1b.tile([128, 128], bf16, name="vtp", tag="vtp")
                        nc.tensor.transpose(vtp[:], vtb[:, i * 128:(i + 1) * 128], idb[:])
                        nc.vector.tensor_copy(vext[ti][:, 0:128], vtp[:])
                        nc.vector.tensor_copy(vext[ti][:, 128:129], ones1[:])

            # ---- phase 2: attention ----
            with (
                tc.tile_pool(name="p2", bufs=17) as p2,
                tc.tile_pool(name="p2s", bufs=2) as p2s,
                tc.tile_pool(name="ps2", bufs=2, space="PSUM") as ps2,
            ):
                for h in range(QPG):
                    for qb in range(TB):
                        qs = slice(qb * 512, (qb + 1) * 512)
                        nkt = 4 * qb + 4  # tk tiles 0 .. 4qb+3
                        pt = []
                        for tk in range(nkt):
                            s_ps = ps2.tile([128, 512], f32, name="s_ps", tag="s_ps", bufs=3)
                            nc.tensor.matmul(
                                s_ps[:], kt[:, tk * 128:(tk + 1) * 128], qt[h][:, qs],
                                start=True, stop=True,
                            )
                            p_t = p2.tile([128, 512], bf16, name="pt", tag="pt")
                            nc.scalar.activation(p_t[:], s_ps[:], EXP, scale=ISD)
                            j = tk - 4 * qb
                            if j >= 0:
                                nc.vector.tensor_tensor(p_t[:], p_t[:], msk[j][:], MULT)
                            pt.append(p_t)
                        for j in range(4):
                            tt = 4 * qb + j
                            o_ps = ps2.tile([128, 129], f32, name="o_ps", tag="o_ps")
                            for tk in range(tt + 1):
                                nc.tensor.matmul(
                                    o_ps[:], pt[tk][:, j * 128:(j + 1) * 128],
                                    vext[tk][:, 0:129],
                                    start=(tk == 0), stop=(tk == tt),
                                )
                            r_sb = p2s.tile([128, 1], f32, name="r", tag="r")
                            nc.vector.reciprocal(r_sb[:], o_ps[:, 128:129])
                            a_sb = p2s.tile([128, 128], f32, name="a", tag="a")
                            nc.scalar.activation(a_sb[:], o_ps[:, 0:128], COPY, scale=r_sb[:])
                            at_ps = ps2.tile([128, 128], f32, name="at_ps", tag="at_ps")
                            nc.tensor.transpose(at_ps[:], a_sb[:], idf[:])
                            nc.vector.tensor_copy(at[h][:, tt * 128:(tt + 1) * 128], at_ps[:])

            # ---- phase 3: output projection (partial) ----
            with (
                tc.tile_pool(name="p3", bufs=2) as p3,
                tc.tile_pool(name="ps3", bufs=4, space="PSUM") as ps3,
            ):
                for ti in range(TT):
                    o_sb = p3.tile([128, E], f32, name="osb", tag="osb")
                    for eo in range(4):
                        w_ps = ps3.tile([128, 512], f32, name="w_ps", tag="w_ps")
                        for h in range(QPG):
                            nc.tensor.matmul(
                                w_ps[:], at[h][:, ti * 128:(ti + 1) * 128],
                                wo_sb[h][:, eo * 512:(eo + 1) * 512],
                                start=(h == 0), stop=(h == QPG - 1),
                            )
                        nc.scalar.copy(o_sb[:, eo * 512:(eo + 1) * 512], w_ps[:])
                    nc.sync.dma_start(out_d[ti * 128:(ti + 1) * 128, :], o_sb[:])

    nc.compile()
    return nc


def _get_compiled():
    global _compiled
    if _compiled is None:
        _compiled = _build()
    return _compiled


def _host_inputs(x, wq, bq, wkv, bkv, wo):
    import jax.numpy as jnp

    pos = np.arange(T, dtype=np.float32)[:, None]
    i = np.arange(0, D, 2, dtype=np.float32)
    inv = np.exp(-(np.log(10000.0) * i / D))
    ang = pos * inv
    pe = np.zeros((T, D), np.float32)
    pe[:, 0::2] = np.sin(ang)
    pe[:, 1::2] = np.cos(ang)
    pet = np.ascontiguousarray(pe.T)

    # causal masks for the 4 diagonal tiles of a 512-wide tq block:
    # mask_j[p, c] = 1 if c >= 128*j + p
    c = np.arange(512)[None, :]
    p = np.arange(128)[:, None]
    msk = np.stack([(c >= 128 * j + p) for j in range(4)]).astype(np.float32)
    msk = np.asarray(jnp.asarray(msk, dtype=jnp.bfloat16))

    idf = np.eye(128, dtype=np.float32)
    idb = np.asarray(jnp.asarray(idf, dtype=jnp.bfloat16))
    ones1 = np.asarray(jnp.ones((128, 1), dtype=jnp.bfloat16))

    xts = [np.ascontiguousarray(x[b].T) for b in range(B)]
    in_maps = []
    for core in range(8):
        b, g = divmod(core, G)
        in_maps.append({
            "xt": xts[b],
            "wq": np.ascontiguousarray(wq[:, g * NQ:(g + 1) * NQ]),
            "wkv": np.ascontiguousarray(wkv[:, g * NKV:(g + 1) * NKV]),
            "wo": np.ascontiguousarray(wo[g * NQ:(g + 1) * NQ, :]),
            "pet": pet,
            "bq": np.ascontiguousarray(bq[g * NQ:(g + 1) * NQ].reshape(QPG, D).T),
            "bk": np.ascontiguousarray(bkv[g * NKV:g * NKV + D].reshape(D, 1)),
            "bv": np.ascontiguousarray(bkv[g * NKV + D:(g + 1) * NKV].reshape(D, 1)),
            "msk": msk,
            "idf": idf,
            "idb": idb,
            "ones1": ones1,
        })
    return in_maps


def run(x, wq, bq, wkv, bkv, wo, trace=False):
    from concourse.bass_utils import run_bass_kernel_spmd

    nc = _get_compiled()
    in_maps = _host_inputs(
        np.asarray(x, np.float32), np.asarray(wq, np.float32),
        np.asarray(bq, np.float32), np.asarray(wkv, np.float32),
        np.asarray(bkv, np.float32), np.asarray(wo, np.float32),
    )
    res = run_bass_kernel_spmd(nc, in_maps, core_ids=list(range(8)), trace=trace)
    out = np.zeros((B, T, E), np.float32)
    for core in range(8):
        b = core // G
        out[b] += res.results[core]["out"]
    return out, res


def kernel(x, wq, bq, wkv, bkv, wo):
    out, _ = run(x, wq, bq, wkv, bkv, wo, trace=False)
    return out
